# revision 7
# baseline (speedup 1.0000x reference)
"""DeepseekV3 MoE layer on 8 trn2 NeuronCores (expert-parallel).

v3 strategy (baseline ~326us HW, v2 ~275us):
* The baseline was DMA-streaming-bound: ~87MB HBM traffic per core at
  ~330GB/s sustained.  Traffic is cut to ~65MB by storing a tunable
  fraction of the expert weights in fp8 e3m4 (4-bit mantissa) with
  per-row scales, mixed into the same accumulation loops (the PE allows
  fp8 stationary x fp16 moving at full speed).  Scale folding is free:
    - gate de-scale rides the SiLU activation's per-partition scale,
    - up de-scale folds into w_down's columns (host-side),
    - down de-scale rides the output eviction's tensor_scalar_mul.
* All 16-bit tensors are fp16 (not bf16): 4x finer mantissa, same speed,
  which buys error budget for the fp8 tiles (sim ~0.015 = HW 0.015).
* Down-projections are reoriented to [h, token] (tokens = moving cols),
  so PE cost is proportional to the exact token count instead of padded
  512-wide tiles; the per-token combine weights move into the host-side
  scatter-add (output is emitted transposed, in per-(slot,h) blocks).
* All chunked tensors are laid out k-major in DRAM so every DMA moves a
  fully contiguous block; outputs are written as contiguous [128 x cap]
  blocks (host reassembles) to keep store descriptors fat.
* The shared-expert gate/up runs k-outer over chunked input DMAs so the
  PE starts ~3us into the kernel; shared-expert down-projection pieces
  are interleaved into routed slots 1..3 to fill DMA-wait gaps.
"""

import numpy as np
import ml_dtypes

import concourse.bass as bass
import concourse.mybir as mybir
import concourse.tile as tile
from concourse.bass_utils import run_bass_kernel_spmd

F16 = np.float16
E3M4 = ml_dtypes.float8_e3m4
Q8TGT = 15.5 * 0.97          # e3m4 max with rounding headroom

# ---- problem constants (fixed by the spec) ----
E, G, EPG, TKG, TOPK = 32, 8, 4, 4, 4
H, I, SI, SCALE = 2048, 1408, 2816, 2.5
T = 1024
NCORES = 8
EPC = E // NCORES          # experts per core = 4
KH = H // 128              # 16 contraction chunks over H
MI = I // 128              # 11 tiles over I
SIL = SI // NCORES         # 352 local shared-intermediate
SIP = 384                  # padded to 3*128
KSI = SIP // 128           # 3
HT16 = H // 128            # 16 output h-chunks

# fp8 tile counts per weight tensor (of MI=11 i-chunks); rest are fp16
N8G, N8U, N8D = 6, 6, 8

_STATE: dict = {}

_TPB_ENGINES = {"Pool", "Activation", "PE", "DVE", "SP"}


def _split_multiwait_bir(bir_bytes: bytes) -> bytes:
    """Walrus codegen here accepts at most one sem-wait per TPB
    instruction.  Move excess waits onto single-wait NoOps inserted
    immediately before the instruction on the same engine."""
    import orjson

    bir = orjson.loads(bir_bytes)
    ctr = 0
    for f in bir["functions"]:
        for blk in f["blocks"]:
            out = []
            for inst in blk["instructions"]:
                si = inst.get("sync_info")
                waits = (si or {}).get("on_wait") or []
                if len(waits) > 1 and inst.get("engine") in _TPB_ENGINES:
                    for w in waits[:-1]:
                        ctr += 1
                        out.append({
                            "debug": inst.get("debug", 0),
                            "engine": inst["engine"],
                            "ins": [],
                            "outs": [],
                            "name": f"I-wsplit-{ctr}",
                            "opcode": "NoOp",
                            "sync_info": {"on_update": [], "on_wait": [w]},
                        })
                    si["on_wait"] = waits[-1:]
                out.append(inst)
            blk["instructions"] = out
    return orjson.dumps(bir)


def _patch_tile():
    if _STATE.get("patched"):
        return
    from concourse.tile import ScopedClock, TileContext

    _orig_to_json = bass.Bass.to_json_bytes

    def to_json_bytes_split(self):
        return _split_multiwait_bir(_orig_to_json(self))

    bass.Bass.to_json_bytes = to_json_bytes_split

    def _drain_and_barrier_split(self, tick_clock, wait_clock):
        probe = self.nc.sync.nop(nofuse=True)
        wait_clock.add_sem_waits(
            probe.ins, ScopedClock({None: tick_clock.global_clock})
        )
        waits = list(probe.ins.sync_info.on_wait) if probe.ins.sync_info else []
        if probe.ins.sync_info:
            probe.ins.sync_info.on_wait = waits[:1]
            for w in waits[1:]:
                n2 = self.nc.sync.nop(nofuse=True)
                si = n2.ins.sync_info
                if si is None:
                    n2.ins.sync_info = mybir.SyncInfo(on_wait=[w], on_update=[])
                else:
                    si.on_wait = [w]
        self.nc.sync.drain()
        self.nc.all_engine_barrier()
        assert self.sems is not None
        popped = self.nc._tile_sem_poison_stack.pop()
        assert popped is self._sem_poison
        self.nc.clear_and_free_semaphores(list(self.sems.allocated().values()))
        self.nc.all_engine_barrier()

    TileContext._drain_and_barrier = _drain_and_barrier_split
    _STATE["patched"] = True


# --------------------------------------------------------------------
# host routing — exact numpy mirror of the reference gate
# --------------------------------------------------------------------
def _gate_host(x, gate_weight, bias):
    Tn = x.shape[0]
    logits = x @ gate_weight.T
    scores = 1.0 / (1.0 + np.exp(-logits))
    sfc = scores + bias[None, :]
    gs = sfc.reshape(Tn, G, EPG)
    top2 = np.sort(gs, axis=-1)[:, :, -2:].sum(-1)
    grp_idx = np.argsort(-top2, axis=-1, kind="stable")[:, :TKG]
    gmask = np.zeros((Tn, G), bool)
    gmask[np.arange(Tn)[:, None], grp_idx] = True
    smask = np.repeat(gmask, EPG, axis=1)
    tmp = np.where(smask, sfc, 0.0)
    topk_idx = np.argsort(-tmp, axis=-1, kind="stable")[:, :TOPK]
    topk_w = np.take_along_axis(scores, topk_idx, axis=1)
    topk_w = topk_w / (topk_w.sum(-1, keepdims=True) + 1e-20)
    return topk_idx, topk_w * SCALE


# --------------------------------------------------------------------
# device kernel (parameterized by per-slot capacities)
# --------------------------------------------------------------------
def _build_nc(caps):
    _patch_tile()
    nc = bass.Bass("TRN2", target_bir_lowering=False, debug=False, num_devices=1)
    f32, f16, e3 = mybir.dt.float32, mybir.dt.float16, mybir.dt.float8e3
    CT = sum(caps)
    CMX = max(caps)
    assert CMX <= 512
    coff = [sum(caps[:s]) for s in range(EPC)]

    # all chunked tensors are k-major: each [k] slice is one contiguous DMA
    xg = nc.dram_tensor("xg", [KH, 128, CT], f16, kind="ExternalInput").ap()
    xs = nc.dram_tensor("xs", [KH, 128, T], f16, kind="ExternalInput").ap()
    wg8 = nc.dram_tensor("wg8", [EPC, N8G, 128, KH * 128], e3, kind="ExternalInput").ap()
    wg16 = nc.dram_tensor("wg16", [EPC, MI - N8G, 128, KH * 128], f16, kind="ExternalInput").ap()
    wu8 = nc.dram_tensor("wu8", [EPC, N8U, 128, KH * 128], e3, kind="ExternalInput").ap()
    wu16 = nc.dram_tensor("wu16", [EPC, MI - N8U, 128, KH * 128], f16, kind="ExternalInput").ap()
    wd8 = nc.dram_tensor("wd8", [EPC, N8D, 128, H], e3, kind="ExternalInput").ap()
    wd16 = nc.dram_tensor("wd16", [EPC, MI - N8D, 128, H], f16, kind="ExternalInput").ap()
    gsc = nc.dram_tensor("gsc", [128, EPC * MI], f32, kind="ExternalInput").ap()
    dsc = nc.dram_tensor("dsc", [128, EPC * HT16], f32, kind="ExternalInput").ap()
    sg = nc.dram_tensor("sg", [KH, 128, SIP], f16, kind="ExternalInput").ap()
    su = nc.dram_tensor("su", [KH, 128, SIP], f16, kind="ExternalInput").ap()
    sd = nc.dram_tensor("sd", [KSI, 128, H], f16, kind="ExternalInput").ap()
    # outputs as contiguous [128, width] blocks; host reassembles
    yrs = [nc.dram_tensor(f"yr{s}", [HT16, 128, caps[s]], f16,
                          kind="ExternalOutput").ap() for s in range(EPC)]
    ys = nc.dram_tensor("ys", [2, HT16, 128, 512], f16, kind="ExternalOutput").ap()

    SILU = mybir.ActivationFunctionType.Silu

    with tile.TileContext(nc) as tc:
        with tc.tile_pool(name="main", bufs=1) as pool, \
             tc.tile_pool(name="psum", bufs=1, space="PSUM") as pp:
            # ---- resident SBUF tiles ----
            gsc_sb = pool.tile([128, EPC * MI], f32, tag="gsc", bufs=1)
            dsc_sb = pool.tile([128, EPC * HT16], f32, tag="dsc", bufs=1)
            sg_sb = pool.tile([128, KH * SIP], f16, tag="sg", bufs=1)
            su_sb = pool.tile([128, KH * SIP], f16, tag="su", bufs=1)
            xs_sb = pool.tile([128, KH * T], f16, tag="xs", bufs=1)
            xg_sb = pool.tile([128, KH * CT], f16, tag="xg", bufs=1)
            as_sb = pool.tile([128, KSI * T], f16, tag="as", bufs=1)
            sd_sb = pool.tile([128, KSI * H], f16, tag="sd", bufs=1)

            # ---- DMA: scales + shared-expert inputs chunked per k ----
            nc.sync.dma_start(gsc_sb[:], gsc[:])
            nc.sync.dma_start(dsc_sb[:], dsc[:])
            for k in range(KH):
                nc.sync.dma_start(sg_sb[:, k * SIP:(k + 1) * SIP], sg[k])
                nc.sync.dma_start(su_sb[:, k * SIP:(k + 1) * SIP], su[k])
                nc.sync.dma_start(xs_sb[:, k * T:(k + 1) * T], xs[k])
            for k in range(KH):
                nc.sync.dma_start(xg_sb[:, k * CT:(k + 1) * CT], xg[k])

            # weight-stream DMA helper: rotating tile pools per dtype
            def load_gu(which, s, m):
                if which == "g":
                    d8, d16, n8 = wg8, wg16, N8G
                else:
                    d8, d16, n8 = wu8, wu16, N8U
                if m < n8:
                    t = pool.tile([128, KH * 128], e3, tag=f"w{which}8", bufs=6,
                                  name=f"w{which}8_{s}_{m}")
                    nc.sync.dma_start(t[:], d8[s, m])
                else:
                    t = pool.tile([128, KH * 128], f16, tag=f"w{which}16", bufs=3,
                                  name=f"w{which}16_{s}_{m}")
                    nc.sync.dma_start(t[:], d16[s, m - n8])
                return t

            def load_wd(s, k2):
                if k2 < N8D:
                    t = pool.tile([128, H], e3, tag="wd8", bufs=N8D + 3,
                                  name=f"wd8_{s}_{k2}")
                    nc.sync.dma_start(t[:], wd8[s, k2])
                else:
                    t = pool.tile([128, H], f16, tag="wd16", bufs=(MI - N8D) + 1,
                                  name=f"wd16_{s}_{k2}")
                    nc.sync.dma_start(t[:], wd16[s, k2 - N8D])
                return t

            # DMA issue order: slot0 g/u, wd0, sd, slot1 g/u, wd1, slot2...
            wgu_tiles = {}
            wd_tiles = {}
            for s in range(EPC):
                for m in range(MI):
                    wgu_tiles[("g", s, m)] = load_gu("g", s, m)
                    wgu_tiles[("u", s, m)] = load_gu("u", s, m)
                for k2 in range(MI):
                    wd_tiles[(s, k2)] = load_wd(s, k2)
                if s == 0:
                    for k in range(KSI):
                        nc.sync.dma_start(sd_sb[:, k * H:(k + 1) * H], sd[k])

            # ================= PE program =================
            # ---- A(nt0): shared gate/up, k-outer (JIT chunks), 6 psums ----
            spg = [pp.tile([128, 512], f32, tag="ps", bufs=8, name=f"spg{m}")
                   for m in range(KSI)]
            spu = [pp.tile([128, 512], f32, tag="ps", bufs=8, name=f"spu{m}")
                   for m in range(KSI)]
            for k in range(KH):
                for m in range(KSI):
                    nc.tensor.matmul(
                        spg[m][:],
                        sg_sb[:, k * SIP + m * 128: k * SIP + (m + 1) * 128],
                        xs_sb[:, k * T: k * T + 512],
                        start=(k == 0), stop=(k == KH - 1))
                    nc.tensor.matmul(
                        spu[m][:],
                        su_sb[:, k * SIP + m * 128: k * SIP + (m + 1) * 128],
                        xs_sb[:, k * T: k * T + 512],
                        start=(k == 0), stop=(k == KH - 1))
            for m in range(KSI):
                sil = pool.tile([128, 512], f32, tag="sil", bufs=2, name=f"ssil0_{m}")
                nc.scalar.activation(sil[:], spg[m][:], SILU)
                nc.vector.tensor_mul(
                    as_sb[:, m * T: m * T + 512], sil[:], spu[m][:])

            # ---- A(nt1): shared gate/up second half, m-outer ----
            for m in range(KSI):
                pg = pp.tile([128, 512], f32, tag="ps", bufs=8, name=f"s1pg{m}")
                pu = pp.tile([128, 512], f32, tag="ps", bufs=8, name=f"s1pu{m}")
                for k in range(KH):
                    nc.tensor.matmul(
                        pg[:],
                        sg_sb[:, k * SIP + m * 128: k * SIP + (m + 1) * 128],
                        xs_sb[:, k * T + 512: k * T + 1024],
                        start=(k == 0), stop=(k == KH - 1))
                for k in range(KH):
                    nc.tensor.matmul(
                        pu[:],
                        su_sb[:, k * SIP + m * 128: k * SIP + (m + 1) * 128],
                        xs_sb[:, k * T + 512: k * T + 1024],
                        start=(k == 0), stop=(k == KH - 1))
                sil = pool.tile([128, 512], f32, tag="sil", bufs=2, name=f"ssil1_{m}")
                nc.scalar.activation(sil[:], pg[:], SILU)
                nc.vector.tensor_mul(
                    as_sb[:, m * T + 512: m * T + 1024], sil[:], pu[:])

            # ---- shared down pieces (interleaved into routed slots 1..3) ----
            def shared_down_piece(tt, h):
                py = pp.tile([128, 512], f32, tag="ps", bufs=8, name=f"pyC{tt}_{h}")
                for k in range(KSI):
                    nc.tensor.matmul(
                        py[:],
                        sd_sb[:, k * H + h * 128: k * H + (h + 1) * 128],
                        as_sb[:, k * T + tt * 512: k * T + (tt + 1) * 512],
                        start=(k == 0), stop=(k == KSI - 1))
                yo = pool.tile([128, 512], f16, tag="yoC", bufs=3, name=f"yoC{tt}_{h}")
                nc.vector.tensor_copy(yo[:], py[:])
                nc.gpsimd.dma_start(ys[tt, h], yo[:])

            cpieces = [(tt, h) for tt in range(2) for h in range(HT16)]
            cpos = 0

            # ---- routed slots ----
            for s in range(EPC):
                cap = caps[s]
                a_sb = pool.tile([128, MI * CMX], f16, tag="a", bufs=2, name=f"a{s}")
                # gate/up
                for m in range(MI):
                    wgt = wgu_tiles[("g", s, m)]
                    wut = wgu_tiles[("u", s, m)]
                    pg = pp.tile([128, cap], f32, tag="ps", bufs=8,
                                 padded_shape=[128, 512], name=f"rpg{s}_{m}")
                    pu = pp.tile([128, cap], f32, tag="ps", bufs=8,
                                 padded_shape=[128, 512], name=f"rpu{s}_{m}")
                    for k in range(KH):
                        nc.tensor.matmul(
                            pg[:], wgt[:, k * 128:(k + 1) * 128],
                            xg_sb[:, k * CT + coff[s]: k * CT + coff[s] + cap],
                            start=(k == 0), stop=(k == KH - 1))
                    for k in range(KH):
                        nc.tensor.matmul(
                            pu[:], wut[:, k * 128:(k + 1) * 128],
                            xg_sb[:, k * CT + coff[s]: k * CT + coff[s] + cap],
                            start=(k == 0), stop=(k == KH - 1))
                    sil = pool.tile([128, cap], f32, tag="sil", bufs=2,
                                    padded_shape=[128, 512], name=f"sil{s}_{m}")
                    nc.scalar.activation(sil[:], pg[:], SILU,
                                         scale=gsc_sb[:, s * MI + m: s * MI + m + 1])
                    nc.vector.tensor_mul(
                        a_sb[:, m * cap:(m + 1) * cap], sil[:], pu[:])
                # down-projection, reoriented [h, tok]
                for h in range(HT16):
                    py = pp.tile([128, cap], f32, tag="ps", bufs=8,
                                 padded_shape=[128, 512], name=f"py{s}_{h}")
                    for k2 in range(MI):
                        nc.tensor.matmul(
                            py[:],
                            wd_tiles[(s, k2)][:, h * 128:(h + 1) * 128],
                            a_sb[:, k2 * cap: k2 * cap + cap],
                            start=(k2 == 0), stop=(k2 == MI - 1))
                    yo = pool.tile([128, cap], f16, tag="yo", bufs=3,
                                   padded_shape=[128, 512], name=f"yo{s}_{h}")
                    nc.vector.tensor_scalar_mul(
                        yo[:], py[:],
                        dsc_sb[:, s * HT16 + h: s * HT16 + h + 1])
                    nc.gpsimd.dma_start(yrs[s][h], yo[:])
                    # interleave shared-down pieces into slots 1..3
                    if s >= 1 and cpos < len(cpieces):
                        shared_down_piece(*cpieces[cpos])
                        cpos += 1
                        if s >= 2 and cpos < len(cpieces) and (h % 2 == 1):
                            shared_down_piece(*cpieces[cpos])
                            cpos += 1
            while cpos < len(cpieces):
                shared_down_piece(*cpieces[cpos])
                cpos += 1

    return nc


def _get_nc(caps):
    key = ("nc", tuple(caps), (N8G, N8U, N8D))
    if key not in _STATE:
        _STATE[key] = _build_nc(caps)
    return _STATE[key]


# --------------------------------------------------------------------
# host packing + quantization
# --------------------------------------------------------------------
def _pack_gu_tile(ws_tile):
    # ws_tile: [128 i, H] (already scaled + cast) -> [128, KH*128] with
    # [p, k*128+c] = ws_tile[c, k*128+p]
    return np.ascontiguousarray(
        ws_tile.reshape(128, KH, 128).transpose(2, 1, 0)).reshape(128, KH * 128)


def _quant_expert(wg_e, wu_e, wd_e):
    """Quantize one expert: returns dict of packed device arrays."""
    out = {}
    mxg = np.maximum(np.abs(wg_e).max(axis=1), 1e-30)
    sgr = (mxg / Q8TGT).astype(np.float32)
    wsg = wg_e / sgr[:, None]
    out["wg8"] = np.stack([
        _pack_gu_tile(wsg[m * 128:(m + 1) * 128].astype(E3M4))
        for m in range(N8G)])
    out["wg16"] = np.stack([
        _pack_gu_tile(wsg[m * 128:(m + 1) * 128].astype(F16))
        for m in range(N8G, MI)])
    out["gsc"] = sgr.reshape(MI, 128).T.copy()      # [128, MI]
    mxu = np.maximum(np.abs(wu_e).max(axis=1), 1e-30)
    sur = (mxu / Q8TGT).astype(np.float32)
    wsu = wu_e / sur[:, None]
    out["wu8"] = np.stack([
        _pack_gu_tile(wsu[m * 128:(m + 1) * 128].astype(E3M4))
        for m in range(N8U)])
    out["wu16"] = np.stack([
        _pack_gu_tile(wsu[m * 128:(m + 1) * 128].astype(F16))
        for m in range(N8U, MI)])
    # down: fold sur into columns, per-row(h) scale over the fp8 chunks
    wde = wd_e * sur[None, :]
    if N8D > 0:
        sdr = (np.maximum(np.abs(wde[:, :N8D * 128]).max(axis=1), 1e-30)
               / Q8TGT).astype(np.float32)
    else:
        sdr = np.ones(H, np.float32)
    wq = wde / sdr[:, None]
    out["wd8"] = np.stack([
        np.ascontiguousarray(wq[:, k2 * 128:(k2 + 1) * 128].T).astype(E3M4)
        for k2 in range(N8D)])
    out["wd16"] = np.stack([
        np.ascontiguousarray(wq[:, k2 * 128:(k2 + 1) * 128].T).astype(F16)
        for k2 in range(N8D, MI)])
    out["dsc"] = sdr.reshape(HT16, 128).T.copy()    # [128, 16]
    return out


def _weight_packs(inp):
    key = tuple(inp[k].ctypes.data for k in
                ("w_gate", "w_up", "w_down", "shared_w_gate",
                 "shared_w_up", "shared_w_down")) + (N8G, N8U, N8D)
    cached = _STATE.get("wpack")
    if cached is not None and cached[0] == key:
        return cached[1]

    wg = inp["w_gate"]; wu = inp["w_up"]; wd = inp["w_down"]
    packs = {"experts": [_quant_expert(wg[e], wu[e], wd[e]) for e in range(E)]}

    sgT = inp["shared_w_gate"].astype(F16).T    # [H, SI]
    suT = inp["shared_w_up"].astype(F16).T
    sdT = inp["shared_w_down"].astype(F16).T    # [SI, H]
    sg_l, su_l, sd_l = [], [], []
    for c in range(NCORES):
        sg_pad = np.zeros((H, SIP), F16)
        sg_pad[:, :SIL] = sgT[:, c * SIL:(c + 1) * SIL]
        su_pad = np.zeros((H, SIP), F16)
        su_pad[:, :SIL] = suT[:, c * SIL:(c + 1) * SIL]
        sd_pad = np.zeros((SIP, H), F16)
        sd_pad[:SIL] = sdT[c * SIL:(c + 1) * SIL]
        sg_l.append(np.ascontiguousarray(sg_pad.reshape(KH, 128, SIP)))
        su_l.append(np.ascontiguousarray(su_pad.reshape(KH, 128, SIP)))
        sd_l.append(np.ascontiguousarray(sd_pad.reshape(KSI, 128, H)))
    packs["sg"], packs["su"], packs["sd"] = sg_l, su_l, sd_l
    _STATE["wpack"] = (key, packs)
    return packs


def kernel(**inputs) -> np.ndarray:
    inp = {k: np.ascontiguousarray(np.asarray(v), dtype=np.float32)
           for k, v in inputs.items()}
    x = inp["hidden_states"].reshape(-1, H)

    topk_idx, topk_w = _gate_host(
        x, inp["gate_weight"], inp["e_score_correction_bias"])

    idx_lists, wt_lists, counts = [], [], []
    for e in range(E):
        tok, slot = np.nonzero(topk_idx == e)
        idx_lists.append(tok)
        wt_lists.append(topk_w[tok, slot])
        counts.append(len(tok))
    counts = np.asarray(counts)

    # assign experts to (core, slot) by sorted load; slot capacity =
    # rank-group max rounded up to 8 (min 16)
    order = np.argsort(-counts, kind="stable")
    assign = np.empty((NCORES, EPC), np.int64)
    caps = []
    for s in range(EPC):
        grp = order[s * NCORES:(s + 1) * NCORES]
        assign[:, s] = grp
        caps.append(max(16, int(-(-int(counts[grp].max()) // 8) * 8)))
    caps = tuple(caps)
    CT = sum(caps)
    coff = [sum(caps[:s]) for s in range(EPC)]

    x16 = x.astype(F16)
    xsT = np.ascontiguousarray(x16.T)                       # [H, T]
    xs_pack = np.ascontiguousarray(xsT.reshape(KH, 128, T))
    packs = _weight_packs(inp)

    in_maps = []
    for c in range(NCORES):
        xga = np.zeros((H, CT), F16)
        wg8_a = np.empty((EPC, N8G, 128, KH * 128), E3M4)
        wg16_a = np.empty((EPC, MI - N8G, 128, KH * 128), F16)
        wu8_a = np.empty((EPC, N8U, 128, KH * 128), E3M4)
        wu16_a = np.empty((EPC, MI - N8U, 128, KH * 128), F16)
        wd8_a = np.empty((EPC, N8D, 128, H), E3M4)
        wd16_a = np.empty((EPC, MI - N8D, 128, H), F16)
        gsc_a = np.empty((128, EPC * MI), np.float32)
        dsc_a = np.empty((128, EPC * HT16), np.float32)
        for s in range(EPC):
            e = int(assign[c, s])
            idx = idx_lists[e]
            xga[:, coff[s]:coff[s] + len(idx)] = x16[idx].T
            pe = packs["experts"][e]
            wg8_a[s] = pe["wg8"]; wg16_a[s] = pe["wg16"]
            wu8_a[s] = pe["wu8"]; wu16_a[s] = pe["wu16"]
            wd8_a[s] = pe["wd8"]; wd16_a[s] = pe["wd16"]
            gsc_a[:, s * MI:(s + 1) * MI] = pe["gsc"]
            dsc_a[:, s * HT16:(s + 1) * HT16] = pe["dsc"]
        in_maps.append({
            "xg": np.ascontiguousarray(xga.reshape(KH, 128, CT)),
            "xs": xs_pack,
            "wg8": wg8_a, "wg16": wg16_a,
            "wu8": wu8_a, "wu16": wu16_a,
            "wd8": wd8_a, "wd16": wd16_a,
            "gsc": gsc_a, "dsc": dsc_a,
            "sg": packs["sg"][c], "su": packs["su"][c], "sd": packs["sd"][c],
        })

    nc = _get_nc(caps)
    _STATE["last_in_maps"] = in_maps
    _STATE["last_caps"] = caps
    last_exc = None
    for _attempt in range(3):
        try:
            res = run_bass_kernel_spmd(nc, in_maps, core_ids=list(range(NCORES)))
            break
        except Exception as exc:  # noqa: BLE001
            last_exc = exc
            import time as _time
            _time.sleep(5.0)
    else:
        raise last_exc

    out = np.zeros((T, H), np.float32)
    for c in range(NCORES):
        ysb = np.asarray(res.results[c]["ys"], np.float32)   # [2,16,128,512]
        out += ysb.transpose(0, 3, 1, 2).reshape(T, H)
    for c in range(NCORES):
        for s in range(EPC):
            e = int(assign[c, s])
            idx = idx_lists[e]
            n = len(idx)
            if n:
                yb = np.asarray(res.results[c][f"yr{s}"], np.float32)  # [16,128,cap]
                yT = yb.reshape(H, caps[s])[:, :n]            # [H, n]
                out[idx] += (yT * wt_lists[e][None, :]).T

    return out.reshape(1, T, H).astype(np.float32)
